# revision 9
# baseline (speedup 1.0000x reference)
"""Trainium2 Bass kernel for nn_Interpolator — grid accumulation, v3.

Reference (N=32768 obs, R=2048 sorted ref timesteps, ninp=64, a=50):
    Ks[r,n] = exp(-a(ref_r - t_n)^2)*mask + EPS,  Kc same with 10a
    lam_s = Ks@onehot + EPS, num_s = Ks@(onehot*v), likewise coarse
    lam = lam_s/R; cross = (num_s@rho)/rowsum(lam_s); coarse = num_c/lam_c
    out = concat([lam, cross, coarse-cross], -1)   [1, R, 192]

The four segment-sums are sums of Gaussians in r (sigma >= 0.032), so we
accumulate them on a uniform G=128 grid (16x less exp/matmul work than
evaluating at all 2048 ref positions) and Catmull-Rom-interpolate to the
ref positions with one small PE matmul (~3e-4 interp error).

Obs axis sharded 8 ways. comb = [onehot*mask | onehot*mask*v] is host-
precomputed in bf16 and DMA'd. Per 128-obs chunk: one DVE op builds
X = g^2 - 2tg, two ACT exps (per-partition bias -a t^2) write both
kernel slabs bf16, one bf16 matmul accumulates all 4 sums into half a
PSUM bank. One 128KB AllReduce (Shared output) combines shards; each
core then interpolates/finishes only its own 256 ref rows via its
per-core W slice and writes [192, 256]; the host transposes and
concatenates the slices.
"""

import os
import sys

import numpy as np

sys.path.insert(0, "/opt/trn_rl_repo")

import concourse.bass as bass
import concourse.tile as tile
from concourse import bacc, mybir
from concourse.bass import _add_dep_helper
from concourse.masks import make_identity

# The image's antenv package lacks axon_hooks (NTFF profiling registry);
# register one so trace=True can profile HW exec time. Harmless if unused.
try:
    import antenv.axon_hooks  # noqa: F401
except ImportError:
    import types as _types

    _m = _types.ModuleType("antenv.axon_hooks")
    _m._hook = None

    def _set_hook(hook):
        _m._hook = hook

    def _get_hook():
        if _m._hook is None:
            try:
                from trn_agent_boot.trn_boot import _ntff_profile_via_ctypes

                _m._hook = _ntff_profile_via_ctypes("/opt/axon/libaxon_pjrt.so")
            except Exception:
                _m._hook = None
        return _m._hook

    _m.set_axon_ntff_profile_hook = _set_hook
    _m.get_axon_ntff_profile_hook = _get_hook
    sys.modules["antenv.axon_hooks"] = _m
    try:
        import antenv

        antenv.axon_hooks = _m
    except ImportError:
        pass

F32 = mybir.dt.float32
BF16 = mybir.dt.bfloat16
Alu = mybir.AluOpType
Act = mybir.ActivationFunctionType

N = 32768
R = 2048
NI = 64
M = 8
ND = N // M          # 4096 obs per core
P = 128
NCHUNK = ND // P     # 32
G = 128              # grid points (= matmul contraction limit)
RS = R // M          # 256 ref rows finished per core
EPS = 1e-7
K_SCALE = 10.0


def build_program(alpha: float):
    nc = bacc.Bacc("TRN2")
    patch_waits = []

    s_in = nc.declare_dram_parameter("s", [ND, 3], F32, isOutput=False)
    comb_in = nc.declare_dram_parameter(
        "comb", [ND, 2 * NI], BF16, isOutput=False
    )
    grid_in = nc.declare_dram_parameter("grid", [G], F32, isOutput=False)
    grid2_in = nc.declare_dram_parameter("grid2", [G], F32, isOutput=False)
    rho_in = nc.declare_dram_parameter("rho", [NI, NI], F32, isOutput=False)
    # corr[0:64] = EPS*(cnt+1); corr[64:128] = EPS*sv  (per-dim EPS pads)
    corr_in = nc.declare_dram_parameter("corr", [P, 1], F32, isOutput=False)
    # per-core W slice: full grid rows x this core's 256 ref columns
    wd_in = nc.declare_dram_parameter("wd", [G, RS], F32, isOutput=False)
    # output slice, quantity-major; host transposes to [RS, 192]
    out_t = nc.declare_dram_parameter("out", [3 * NI, RS], F32, isOutput=True)

    with tile.TileContext(nc) as tc:
        with (
            tc.tile_pool(name="consts", bufs=1) as consts,
            tc.tile_pool(name="dram", bufs=1, space="DRAM") as dram,
        ):
            # ---------------- constants ----------------
            gridrow = consts.tile([1, G], F32)
            nc.sync.dma_start(out=gridrow[:], in_=grid_in[None, :])
            grid2row = consts.tile([1, G], F32)
            nc.sync.dma_start(out=grid2row[:], in_=grid2_in[None, :])
            sdata = consts.tile([P, NCHUNK, 3], F32)
            nc.sync.dma_start(
                out=sdata[:], in_=s_in[:].rearrange("(c p) k -> p c k", p=P)
            )
            combH = consts.tile([P, NCHUNK, 2 * NI], BF16)
            comb_r = comb_in[:].rearrange("(c p) k -> p c k", p=P)
            for q4 in range(4):
                cs = q4 * (NCHUNK // 4)
                ce = cs + NCHUNK // 4
                nc.sync.dma_start(out=combH[:, cs:ce, :], in_=comb_r[:, cs:ce, :])
            corr_col = consts.tile([P, 1], F32)
            nc.sync.dma_start(out=corr_col[:], in_=corr_in[:])
            rho_sb = consts.tile([NI, NI], F32)
            nc.sync.dma_start(out=rho_sb[:], in_=rho_in[:])
            wd_sb = consts.tile([G, RS], F32)
            nc.sync.dma_start(out=wd_sb[:], in_=wd_in[:])

            ones_row = consts.tile([1, P], F32)
            nc.vector.memset(ones_row, 1.0)
            ones_col = consts.tile([NI, 1], F32)
            nc.vector.memset(ones_col, 1.0)
            identity = consts.tile([P, P], F32)
            make_identity(nc, identity)
            gridrow2 = consts.tile([1, G], F32)
            nc.vector.tensor_copy(out=gridrow2[:], in_=gridrow[:])
            grid2row2 = consts.tile([1, G], F32)
            nc.vector.tensor_copy(out=grid2row2[:], in_=grid2row[:])

            # grid (and grid^2) broadcast to all 128 partitions via PE
            g_bcast = consts.tile([P, G], F32)
            g2_bcast = consts.tile([P, G], F32)
            with tc.tile_pool(name="bps", bufs=2, space="PSUM") as bps:
                pb = bps.tile([P, G], F32, tag="pb")
                nc.tensor.matmul(
                    pb[:], ones_row[0:1, :], gridrow2[0:1, :], start=True, stop=True
                )
                nc.scalar.copy(out=g_bcast[:], in_=pb[:])
                pb2 = bps.tile([P, G], F32, tag="pb")
                nc.tensor.matmul(
                    pb2[:], ones_row[0:1, :], grid2row2[0:1, :], start=True, stop=True
                )
                nc.scalar.copy(out=g2_bcast[:], in_=pb2[:])

            # per-chunk scalars: m2t = -2t, bias_s = -a t^2, bias_c = -10a t^2
            tcol = sdata[:, :, 0]                       # [P, NCHUNK]
            m2t = consts.tile([P, NCHUNK], F32)
            nc.vector.tensor_scalar(
                out=m2t[:], in0=tcol, scalar1=-2.0, scalar2=None, op0=Alu.mult
            )
            t2 = consts.tile([P, NCHUNK], F32)
            nc.vector.tensor_mul(out=t2[:], in0=tcol, in1=tcol)
            bias_s = consts.tile([P, NCHUNK], F32)
            nc.vector.tensor_scalar(
                out=bias_s[:], in0=t2[:], scalar1=-alpha, scalar2=None, op0=Alu.mult
            )
            t9 = consts.tile([P, NCHUNK], F32)
            nc.vector.tensor_scalar(
                out=t9[:], in0=t2[:], scalar1=K_SCALE - 1.0, scalar2=None,
                op0=Alu.mult,
            )

            part = consts.tile([P, 2, G], F32)

            # ---------------- main loop ----------------
            with (
                tc.tile_pool(name="acc", bufs=1, space="PSUM") as accpool,
                tc.tile_pool(name="work", bufs=3) as work,
            ):
                acc = accpool.tile([P, 2 * G], F32, name="acc", tag="acc")

                for c in range(NCHUNK):
                    # xg2[:,0,:] = g^2-2tg;  xg2[:,1,:] = 10*that + 9t^2
                    # so exp(-a*(xg2 + t^2)) gives both kernels in one op
                    xg2 = work.tile([P, 2, G], F32, tag="xg")
                    nc.vector.scalar_tensor_tensor(
                        out=xg2[:, 0, :],
                        in0=g_bcast[:],
                        scalar=m2t[:, c : c + 1],
                        in1=g2_bcast[:],
                        op0=Alu.mult,
                        op1=Alu.add,
                    )
                    nc.gpsimd.tensor_scalar(
                        out=xg2[:, 1, :],
                        in0=xg2[:, 0, :],
                        scalar1=K_SCALE,
                        scalar2=t9[:, c : c + 1],
                        op0=Alu.mult,
                        op1=Alu.add,
                    )
                    kek = work.tile([P, 2 * G], BF16, tag="kek")
                    nc.scalar.activation(
                        out=kek[:], in_=xg2[:], func=Act.Exp,
                        scale=-alpha, bias=bias_s[:, c : c + 1],
                    )
                    nc.tensor.matmul(
                        acc[:, :],
                        combH[:, c, :],
                        kek[:, :],
                        start=(c == 0),
                        stop=(c == NCHUNK - 1),
                    )

                nc.vector.tensor_copy(
                    out=part[:], in_=acc[:].rearrange("p (q g) -> p q g", q=2)
                )

            # ---------- all-gather via relative remote DMA + local reduce ----
            # bcast k sends my partials to core (self XOR k), landing in that
            # core's allgat slot k; 7 bcasts fill every core's slots 1..7.
            allgat = consts.tile([P, M, 2 * G], F32)
            agsem = nc.alloc_semaphore("agsem")
            aglocal = nc.alloc_semaphore("aglocal")
            for k in range(1, M):
                rd = [None] * M
                rd[k] = (0, k)
                nc.gpsimd.remote_dma_broadcast(
                    out_ap=allgat[:, k, :],
                    in_ap=part[:],
                    remote_sem=agsem,
                    local_sem=aglocal,
                    rdests=rd,
                )
            tr = nc.gpsimd.trigger_dma(count=None)

            # ---------------- finishing (own 256-col slice) ----------------
            with (
                tc.tile_pool(name="fin", bufs=1) as fin,
                tc.tile_pool(name="fps", bufs=1, space="PSUM") as fps,
            ):
                s1 = fin.tile([P, 2, G], F32)
                i1 = nc.vector.tensor_add(out=s1[:], in0=part[:],
                                          in1=allgat[:, 1, :].rearrange("p (q g) -> p q g", q=2))
                s2 = fin.tile([P, 2, G], F32)
                i2 = nc.vector.tensor_add(out=s2[:], in0=allgat[:, 2, :].rearrange("p (q g) -> p q g", q=2),
                                          in1=allgat[:, 3, :].rearrange("p (q g) -> p q g", q=2))
                s3 = fin.tile([P, 2, G], F32)
                i3 = nc.gpsimd.tensor_add(out=s3[:], in0=allgat[:, 4, :].rearrange("p (q g) -> p q g", q=2),
                                          in1=allgat[:, 5, :].rearrange("p (q g) -> p q g", q=2))
                s4 = fin.tile([P, 2, G], F32)
                i4 = nc.gpsimd.tensor_add(out=s4[:], in0=allgat[:, 6, :].rearrange("p (q g) -> p q g", q=2),
                                          in1=allgat[:, 7, :].rearrange("p (q g) -> p q g", q=2))
                # Pool-engine adds must stay after the trigger (same queue);
                # DVE adds get a sync dep. The real gate (agsem >= 14, filled
                # by the 7 remote writes) is appended post-schedule below.
                for ii, sync in ((i1, True), (i2, True), (i3, False), (i4, False)):
                    _add_dep_helper(ii.ins, tr.ins, sync=sync,
                                    reason="reduce after rdma trigger")
                    patch_waits.append((ii, agsem))
                u1 = fin.tile([P, 2, G], F32)
                nc.vector.tensor_add(out=u1[:], in0=s1[:], in1=s2[:])
                u2 = fin.tile([P, 2, G], F32)
                nc.gpsimd.tensor_add(out=u2[:], in0=s3[:], in1=s4[:])
                rsres = fin.tile([P, 2, G], F32)
                nc.vector.tensor_add(out=rsres[:], in0=u1[:], in1=u2[:])
                # transpose each kernel's [128 rows, G] -> qt_q [G, 128 rows]
                qts = []
                for q in range(2):
                    qp = fps.tile([G, P], F32, tag=f"qt{q}")
                    nc.tensor.transpose(qp[:], rsres[:, q, :], identity[:])
                    qs = fin.tile([G, P], F32, name=f"qts{q}")
                    nc.scalar.copy(out=qs[:], in_=qp[:])
                    qts.append(qs)

                # interp matmuls: out[k, r] = sum_g qt[g, k] * W[g, r]
                interp = {}
                specs = [
                    ("ls", 0, slice(0, NI)),
                    ("ns", 0, slice(NI, P)),
                    ("lc", 1, slice(0, NI)),
                    ("nq", 1, slice(NI, P)),
                ]
                for nm, q, sl in specs:
                    ip = fps.tile([NI, RS], F32, tag=f"ip_{nm}")
                    nc.tensor.matmul(
                        ip[:], qts[q][:, sl], wd_sb[:, :], start=True, stop=True
                    )
                    sb = fin.tile([NI, RS], F32, name=f"sb_{nm}")
                    cc = corr_col[0:NI, :] if nm in ("ls", "lc") else corr_col[NI:P, :]
                    nc.vector.tensor_scalar(
                        out=sb[:], in0=ip[:], scalar1=cc, scalar2=None, op0=Alu.add
                    )
                    interp[nm] = sb

                ls, ns, lc, nq = (interp[k] for k in ("ls", "ns", "lc", "nq"))

                # D[r] = sum_k ls[k, r] via PE; stack [lc; D] for one reciprocal
                dps = fps.tile([1, RS], F32, tag="ip_ls")
                nc.tensor.matmul(dps[:], ones_col[:], ls[:, :], start=True, stop=True)
                rec = fin.tile([NI, RS], F32)
                nc.vector.reciprocal_approx_fast(out=rec[:], in_=lc[:])
                recd = fin.tile([1, RS], F32)
                nc.vector.reciprocal_approx_fast(out=recd[:], in_=dps[:])
                recd2 = fin.tile([1, RS], F32)
                nc.scalar.copy(out=recd2[:], in_=recd[:])

                lam_out = fin.tile([NI, RS], F32)
                nc.vector.tensor_scalar(
                    out=lam_out[:], in0=ls[:], scalar1=1.0 / R, scalar2=None,
                    op0=Alu.mult,
                )
                coarse = fin.tile([NI, RS], F32)
                nc.vector.tensor_mul(out=coarse[:], in0=nq[:], in1=rec[:])

                crp = fps.tile([NI, RS], F32, tag="ip_ns")
                nc.tensor.matmul(crp[:], rho_sb[:], ns[:, :], start=True, stop=True)
                dbp = fps.tile([NI, RS], F32, tag="ip_lc")
                nc.tensor.matmul(
                    dbp[:], ones_row[0:1, 0:NI], recd2[0:1, :], start=True, stop=True
                )
                dbc = fin.tile([NI, RS], F32)
                nc.scalar.copy(out=dbc[:], in_=dbp[:])
                cross = fin.tile([NI, RS], F32)
                nc.vector.tensor_mul(out=cross[:], in0=crp[:], in1=dbc[:])
                transient = fin.tile([NI, RS], F32)
                nc.vector.tensor_sub(out=transient[:], in0=coarse[:], in1=cross[:])

                nc.sync.dma_start(out=out_t[0:NI, :], in_=lam_out[:])
                nc.sync.dma_start(out=out_t[NI : 2 * NI, :], in_=cross[:])
                nc.sync.dma_start(out=out_t[2 * NI : 3 * NI, :], in_=transient[:])

    import bass_rust as _br

    for w, sem in patch_waits:
        ow = list(w.ins.sync_info.on_wait)
        ow.append(
            _br.SyncWait(
                sync_type="semaphore",
                id=sem.num,
                ant_name=sem.name,
                wait_mode="sem-ge-imm",
                wait_value=2 * (M - 1),
                wait_reg=None,
            )
        )
        w.ins.sync_info.on_wait = ow
    nc.finalize()
    return nc


_prog_cache = {}


def _get_prog(alpha: float):
    key = round(float(alpha), 9)
    if key not in _prog_cache:
        _prog_cache[key] = build_program(float(alpha))
    return _prog_cache[key]


def _catmull_rom(ref, g0, dg, G):
    """Dense [G, R] Catmull-Rom interpolation matrix."""
    u = (ref - g0) / dg
    i = np.floor(u).astype(np.int64)
    f = (u - i).astype(np.float64)
    w = [
        -0.5 * f**3 + f**2 - 0.5 * f,
        1.5 * f**3 - 2.5 * f**2 + 1.0,
        -1.5 * f**3 + 2.0 * f**2 + 0.5 * f,
        0.5 * f**3 - 0.5 * f**2,
    ]
    W = np.zeros((G, ref.shape[0]), np.float64)
    cols = np.arange(ref.shape[0])
    for off, wk in zip((-1, 0, 1, 2), w):
        idx = i + off
        assert idx.min() >= 0 and idx.max() < G
        W[idx, cols] += wk
    return W


last_results = None


def kernel(S, reference_timesteps, alpha, rho):
    global last_results
    import ml_dtypes

    S = np.ascontiguousarray(np.asarray(S, dtype=np.float32))
    ref = np.ascontiguousarray(
        np.asarray(reference_timesteps, dtype=np.float32)
    )
    rho = np.ascontiguousarray(np.asarray(rho, dtype=np.float32))
    a = float(np.asarray(alpha).reshape(-1)[0])

    assert S.shape == (N, 3) and ref.shape == (1, R) and rho.shape == (NI, NI)

    refd = ref[0].astype(np.float64)
    lo, hi = refd.min(), refd.max()
    dg = (hi - lo) / (G - 5)
    g0 = lo - 2 * dg
    grid = (g0 + dg * np.arange(G)).astype(np.float64)
    W = _catmull_rom(refd, g0, dg, G)

    nc = _get_prog(a)

    t = S[:, 0].astype(np.float64)
    v = S[:, 1].astype(np.float64)
    dims = S[:, 2].astype(np.int32)
    mask = (t > 0).astype(np.float64)
    cnt = np.bincount(dims, minlength=NI).astype(np.float64)
    sv = np.bincount(dims, weights=v, minlength=NI)
    corr = np.concatenate([EPS * (cnt + 1.0), EPS * sv]).astype(np.float32)
    corr = corr.reshape(P, 1)

    # host-precomputed stationary weights [N, 128] in bf16
    comb = np.zeros((N, 2 * NI), np.float32)
    rows = np.arange(N)
    comb[rows, dims] = mask
    comb[rows, NI + dims] = mask * v
    comb = comb.astype(ml_dtypes.bfloat16)

    in_maps = []
    for i in range(M):
        in_maps.append(
            {
                "s": S[i * ND : (i + 1) * ND],
                "comb": comb[i * ND : (i + 1) * ND],
                "grid": grid.astype(np.float32),
                "grid2": (grid * grid).astype(np.float32),
                "rho": rho,
                "corr": corr,
                "wd": np.ascontiguousarray(
                    W[:, i * RS : (i + 1) * RS].astype(np.float32)
                ),
            }
        )

    if os.environ.get("BASS_SIM"):
        # fake_nrt lacks NC-topology FFI; sim just needs identity maps
        import concourse.libnrt as _lnrt
        import concourse.bass_interp as _bi

        _lnrt.get_device_id_to_routing_id_mapping = lambda: {0: 0}
        _lnrt.nc_to_real_nc = lambda d, i: i
        _lnrt.pnc_id_to_device_and_real_nc_index = lambda cid: (cid // 8, cid % 8)
        for _nm in (
            "get_device_id_to_routing_id_mapping",
            "nc_to_real_nc",
            "pnc_id_to_device_and_real_nc_index",
        ):
            if hasattr(_bi, _nm):
                setattr(_bi, _nm, getattr(_lnrt, _nm))
        from concourse.bass_interp import MultiCoreSim

        sim = MultiCoreSim(nc, M)
        for i in range(M):
            for k, val in in_maps[i].items():
                sim.cores[i].tensor(k)[:] = val
        sim.simulate()
        out = np.concatenate(
            [np.array(sim.cores[i].tensor("out")).T for i in range(M)], axis=0
        )
        last_results = None
    else:
        from concourse.bass_utils import run_bass_kernel_spmd

        res = run_bass_kernel_spmd(
            nc,
            in_maps,
            list(range(M)),
            trace=bool(os.environ.get("BASS_TRACE")),
        )
        last_results = res
        out = np.concatenate(
            [np.asarray(res.results[i]["out"]).T for i in range(M)], axis=0
        )

    return np.ascontiguousarray(out).reshape(1, R, 3 * NI).astype(np.float32)


# revision 10
# speedup vs baseline: 61.3726x; 61.3726x over previous
"""Trainium2 Bass kernel for nn_Interpolator — grid accumulation, v3.

Reference (N=32768 obs, R=2048 sorted ref timesteps, ninp=64, a=50):
    Ks[r,n] = exp(-a(ref_r - t_n)^2)*mask + EPS,  Kc same with 10a
    lam_s = Ks@onehot + EPS, num_s = Ks@(onehot*v), likewise coarse
    lam = lam_s/R; cross = (num_s@rho)/rowsum(lam_s); coarse = num_c/lam_c
    out = concat([lam, cross, coarse-cross], -1)   [1, R, 192]

The four segment-sums are sums of Gaussians in r (sigma >= 0.032), so we
accumulate them on a uniform G=128 grid (16x less exp/matmul work than
evaluating at all 2048 ref positions) and Catmull-Rom-interpolate to the
ref positions with one small PE matmul (~3e-4 interp error).

Obs axis sharded 8 ways. comb = [onehot*mask | onehot*mask*v] is host-
precomputed in bf16 and DMA'd. Per 128-obs chunk: one DVE op builds
X = g^2 - 2tg, two ACT exps (per-partition bias -a t^2) write both
kernel slabs bf16, one bf16 matmul accumulates all 4 sums into half a
PSUM bank. One 128KB AllReduce (Shared output) combines shards; each
core then interpolates/finishes only its own 256 ref rows via its
per-core W slice and writes [192, 256]; the host transposes and
concatenates the slices.
"""

import os
import sys

import numpy as np

sys.path.insert(0, "/opt/trn_rl_repo")

import concourse.bass as bass
import concourse.tile as tile
from concourse import bacc, mybir
from concourse.masks import make_identity

# The image's antenv package lacks axon_hooks (NTFF profiling registry);
# register one so trace=True can profile HW exec time. Harmless if unused.
try:
    import antenv.axon_hooks  # noqa: F401
except ImportError:
    import types as _types

    _m = _types.ModuleType("antenv.axon_hooks")
    _m._hook = None

    def _set_hook(hook):
        _m._hook = hook

    def _get_hook():
        if _m._hook is None:
            try:
                from trn_agent_boot.trn_boot import _ntff_profile_via_ctypes

                _m._hook = _ntff_profile_via_ctypes("/opt/axon/libaxon_pjrt.so")
            except Exception:
                _m._hook = None
        return _m._hook

    _m.set_axon_ntff_profile_hook = _set_hook
    _m.get_axon_ntff_profile_hook = _get_hook
    sys.modules["antenv.axon_hooks"] = _m
    try:
        import antenv

        antenv.axon_hooks = _m
    except ImportError:
        pass

F32 = mybir.dt.float32
BF16 = mybir.dt.bfloat16
Alu = mybir.AluOpType
Act = mybir.ActivationFunctionType

N = 32768
R = 2048
NI = 64
M = 8
ND = N // M          # 4096 obs per core
P = 128
NCHUNK = ND // P     # 32
G = 128              # grid points (= matmul contraction limit)
RS = R // M          # 256 ref rows finished per core
EPS = 1e-7
K_SCALE = 10.0


def build_program(alpha: float):
    nc = bacc.Bacc("TRN2")

    s_in = nc.declare_dram_parameter("s", [ND, 3], F32, isOutput=False)
    comb_in = nc.declare_dram_parameter(
        "comb", [ND, 2 * NI], BF16, isOutput=False
    )
    grid_in = nc.declare_dram_parameter("grid", [G], F32, isOutput=False)
    grid2_in = nc.declare_dram_parameter("grid2", [G], F32, isOutput=False)
    rho_in = nc.declare_dram_parameter("rho", [NI, NI], F32, isOutput=False)
    # corr[0:64] = EPS*(cnt+1); corr[64:128] = EPS*sv  (per-dim EPS pads)
    corr_in = nc.declare_dram_parameter("corr", [P, 1], F32, isOutput=False)
    # per-core W slice: full grid rows x this core's 256 ref columns
    wd_in = nc.declare_dram_parameter("wd", [G, RS], F32, isOutput=False)
    # output slice, quantity-major; host transposes to [RS, 192]
    out_t = nc.declare_dram_parameter("out", [3 * NI, RS], F32, isOutput=True)

    with tile.TileContext(nc) as tc:
        with (
            tc.tile_pool(name="consts", bufs=1) as consts,
            tc.tile_pool(name="dram", bufs=1, space="DRAM") as dram,
        ):
            # ---------------- constants ----------------
            gridrow = consts.tile([1, G], F32)
            nc.sync.dma_start(out=gridrow[:], in_=grid_in[None, :])
            grid2row = consts.tile([1, G], F32)
            nc.sync.dma_start(out=grid2row[:], in_=grid2_in[None, :])
            sdata = consts.tile([P, NCHUNK, 3], F32)
            nc.sync.dma_start(
                out=sdata[:], in_=s_in[:].rearrange("(c p) k -> p c k", p=P)
            )
            combH = consts.tile([P, NCHUNK, 2 * NI], BF16)
            comb_r = comb_in[:].rearrange("(c p) k -> p c k", p=P)
            for q4 in range(4):
                cs = q4 * (NCHUNK // 4)
                ce = cs + NCHUNK // 4
                nc.sync.dma_start(out=combH[:, cs:ce, :], in_=comb_r[:, cs:ce, :])
            corr_col = consts.tile([P, 1], F32)
            nc.sync.dma_start(out=corr_col[:], in_=corr_in[:])
            rho_sb = consts.tile([NI, NI], F32)
            nc.sync.dma_start(out=rho_sb[:], in_=rho_in[:])
            wd_sb = consts.tile([G, RS], F32)
            nc.sync.dma_start(out=wd_sb[:], in_=wd_in[:])

            ones_row = consts.tile([1, P], F32)
            nc.vector.memset(ones_row, 1.0)
            ones_col = consts.tile([NI, 1], F32)
            nc.vector.memset(ones_col, 1.0)
            identity = consts.tile([P, P], F32)
            make_identity(nc, identity)
            gridrow2 = consts.tile([1, G], F32)
            nc.vector.tensor_copy(out=gridrow2[:], in_=gridrow[:])
            grid2row2 = consts.tile([1, G], F32)
            nc.vector.tensor_copy(out=grid2row2[:], in_=grid2row[:])

            # grid (and grid^2) broadcast to all 128 partitions via PE
            g_bcast = consts.tile([P, G], F32)
            g2_bcast = consts.tile([P, G], F32)
            with tc.tile_pool(name="bps", bufs=2, space="PSUM") as bps:
                pb = bps.tile([P, G], F32, tag="pb")
                nc.tensor.matmul(
                    pb[:], ones_row[0:1, :], gridrow2[0:1, :], start=True, stop=True
                )
                nc.scalar.copy(out=g_bcast[:], in_=pb[:])
                pb2 = bps.tile([P, G], F32, tag="pb")
                nc.tensor.matmul(
                    pb2[:], ones_row[0:1, :], grid2row2[0:1, :], start=True, stop=True
                )
                nc.scalar.copy(out=g2_bcast[:], in_=pb2[:])

            # per-chunk scalars: m2t = -2t, bias_s = -a t^2, bias_c = -10a t^2
            tcol = sdata[:, :, 0]                       # [P, NCHUNK]
            m2t = consts.tile([P, NCHUNK], F32)
            nc.vector.tensor_scalar(
                out=m2t[:], in0=tcol, scalar1=-2.0, scalar2=None, op0=Alu.mult
            )
            t2 = consts.tile([P, NCHUNK], F32)
            nc.vector.tensor_mul(out=t2[:], in0=tcol, in1=tcol)
            bias_s = consts.tile([P, NCHUNK], F32)
            nc.vector.tensor_scalar(
                out=bias_s[:], in0=t2[:], scalar1=-alpha, scalar2=None, op0=Alu.mult
            )
            t9 = consts.tile([P, NCHUNK], F32)
            nc.vector.tensor_scalar(
                out=t9[:], in0=t2[:], scalar1=K_SCALE - 1.0, scalar2=None,
                op0=Alu.mult,
            )

            part = consts.tile([P, 2, G], F32)

            # ---------------- main loop ----------------
            with (
                tc.tile_pool(name="acc", bufs=1, space="PSUM") as accpool,
                tc.tile_pool(name="work", bufs=3) as work,
            ):
                acc = accpool.tile([P, 2 * G], F32, name="acc", tag="acc")

                for c in range(NCHUNK):
                    # xg2[:,0,:] = g^2-2tg;  xg2[:,1,:] = 10*that + 9t^2
                    # so exp(-a*(xg2 + t^2)) gives both kernels in one op
                    xg2 = work.tile([P, 2, G], F32, tag="xg")
                    nc.vector.scalar_tensor_tensor(
                        out=xg2[:, 0, :],
                        in0=g_bcast[:],
                        scalar=m2t[:, c : c + 1],
                        in1=g2_bcast[:],
                        op0=Alu.mult,
                        op1=Alu.add,
                    )
                    nc.gpsimd.tensor_scalar(
                        out=xg2[:, 1, :],
                        in0=xg2[:, 0, :],
                        scalar1=K_SCALE,
                        scalar2=t9[:, c : c + 1],
                        op0=Alu.mult,
                        op1=Alu.add,
                    )
                    kek = work.tile([P, 2 * G], BF16, tag="kek")
                    nc.scalar.activation(
                        out=kek[:], in_=xg2[:], func=Act.Exp,
                        scale=-alpha, bias=bias_s[:, c : c + 1],
                    )
                    nc.tensor.matmul(
                        acc[:, :],
                        combH[:, c, :],
                        kek[:, :],
                        start=(c == 0),
                        stop=(c == NCHUNK - 1),
                    )

                nc.vector.tensor_copy(
                    out=part[:], in_=acc[:].rearrange("p (q g) -> p q g", q=2)
                )

            # ---------------- all-reduce (Shared out) ----------------
            ar_in = dram.tile([P, 2, G], F32, name="ar_in")
            ar_out = dram.tile([P, 2, G], F32, name="ar_out", addr_space="Shared")
            nc.sync.dma_start(out=ar_in[:], in_=part[:])
            nc.gpsimd.collective_compute(
                "AllReduce",
                Alu.add,
                replica_groups=[list(range(M))],
                ins=[ar_in[:].opt()],
                outs=[ar_out[:].opt()],
            )

            # ---------------- finishing (own 256-col slice) ----------------
            with (
                tc.tile_pool(name="fin", bufs=1) as fin,
                tc.tile_pool(name="fps", bufs=1, space="PSUM") as fps,
            ):
                rsres = fin.tile([P, 2, G], F32)
                nc.sync.dma_start(out=rsres[:], in_=ar_out[:])
                # transpose each kernel's [128 rows, G] -> qt_q [G, 128 rows]
                qts = []
                for q in range(2):
                    qp = fps.tile([G, P], F32, tag=f"qt{q}")
                    nc.tensor.transpose(qp[:], rsres[:, q, :], identity[:])
                    qs = fin.tile([G, P], F32, name=f"qts{q}")
                    nc.scalar.copy(out=qs[:], in_=qp[:])
                    qts.append(qs)

                # interp matmuls: out[k, r] = sum_g qt[g, k] * W[g, r]
                interp = {}
                specs = [
                    ("ls", 0, slice(0, NI)),
                    ("ns", 0, slice(NI, P)),
                    ("lc", 1, slice(0, NI)),
                    ("nq", 1, slice(NI, P)),
                ]
                for nm, q, sl in specs:
                    ip = fps.tile([NI, RS], F32, tag=f"ip_{nm}")
                    nc.tensor.matmul(
                        ip[:], qts[q][:, sl], wd_sb[:, :], start=True, stop=True
                    )
                    sb = fin.tile([NI, RS], F32, name=f"sb_{nm}")
                    cc = corr_col[0:NI, :] if nm in ("ls", "lc") else corr_col[NI:P, :]
                    nc.vector.tensor_scalar(
                        out=sb[:], in0=ip[:], scalar1=cc, scalar2=None, op0=Alu.add
                    )
                    interp[nm] = sb

                ls, ns, lc, nq = (interp[k] for k in ("ls", "ns", "lc", "nq"))

                # D[r] = sum_k ls[k, r] via PE; stack [lc; D] for one reciprocal
                dps = fps.tile([1, RS], F32, tag="ip_ls")
                nc.tensor.matmul(dps[:], ones_col[:], ls[:, :], start=True, stop=True)
                rec = fin.tile([NI, RS], F32)
                nc.vector.reciprocal_approx_fast(out=rec[:], in_=lc[:])
                recd = fin.tile([1, RS], F32)
                nc.vector.reciprocal_approx_fast(out=recd[:], in_=dps[:])
                recd2 = fin.tile([1, RS], F32)
                nc.scalar.copy(out=recd2[:], in_=recd[:])

                lam_out = fin.tile([NI, RS], F32)
                nc.vector.tensor_scalar(
                    out=lam_out[:], in0=ls[:], scalar1=1.0 / R, scalar2=None,
                    op0=Alu.mult,
                )
                coarse = fin.tile([NI, RS], F32)
                nc.vector.tensor_mul(out=coarse[:], in0=nq[:], in1=rec[:])

                crp = fps.tile([NI, RS], F32, tag="ip_ns")
                nc.tensor.matmul(crp[:], rho_sb[:], ns[:, :], start=True, stop=True)
                dbp = fps.tile([NI, RS], F32, tag="ip_lc")
                nc.tensor.matmul(
                    dbp[:], ones_row[0:1, 0:NI], recd2[0:1, :], start=True, stop=True
                )
                dbc = fin.tile([NI, RS], F32)
                nc.scalar.copy(out=dbc[:], in_=dbp[:])
                cross = fin.tile([NI, RS], F32)
                nc.vector.tensor_mul(out=cross[:], in0=crp[:], in1=dbc[:])
                transient = fin.tile([NI, RS], F32)
                nc.vector.tensor_sub(out=transient[:], in0=coarse[:], in1=cross[:])

                nc.sync.dma_start(out=out_t[0:NI, :], in_=lam_out[:])
                nc.sync.dma_start(out=out_t[NI : 2 * NI, :], in_=cross[:])
                nc.sync.dma_start(out=out_t[2 * NI : 3 * NI, :], in_=transient[:])

    nc.finalize()
    return nc


_prog_cache = {}


def _get_prog(alpha: float):
    key = round(float(alpha), 9)
    if key not in _prog_cache:
        _prog_cache[key] = build_program(float(alpha))
    return _prog_cache[key]


def _catmull_rom(ref, g0, dg, G):
    """Dense [G, R] Catmull-Rom interpolation matrix."""
    u = (ref - g0) / dg
    i = np.floor(u).astype(np.int64)
    f = (u - i).astype(np.float64)
    w = [
        -0.5 * f**3 + f**2 - 0.5 * f,
        1.5 * f**3 - 2.5 * f**2 + 1.0,
        -1.5 * f**3 + 2.0 * f**2 + 0.5 * f,
        0.5 * f**3 - 0.5 * f**2,
    ]
    W = np.zeros((G, ref.shape[0]), np.float64)
    cols = np.arange(ref.shape[0])
    for off, wk in zip((-1, 0, 1, 2), w):
        idx = i + off
        assert idx.min() >= 0 and idx.max() < G
        W[idx, cols] += wk
    return W


last_results = None


def kernel(S, reference_timesteps, alpha, rho):
    global last_results
    import ml_dtypes

    S = np.ascontiguousarray(np.asarray(S, dtype=np.float32))
    ref = np.ascontiguousarray(
        np.asarray(reference_timesteps, dtype=np.float32)
    )
    rho = np.ascontiguousarray(np.asarray(rho, dtype=np.float32))
    a = float(np.asarray(alpha).reshape(-1)[0])

    assert S.shape == (N, 3) and ref.shape == (1, R) and rho.shape == (NI, NI)

    refd = ref[0].astype(np.float64)
    lo, hi = refd.min(), refd.max()
    dg = (hi - lo) / (G - 5)
    g0 = lo - 2 * dg
    grid = (g0 + dg * np.arange(G)).astype(np.float64)
    W = _catmull_rom(refd, g0, dg, G)

    nc = _get_prog(a)

    t = S[:, 0].astype(np.float64)
    v = S[:, 1].astype(np.float64)
    dims = S[:, 2].astype(np.int32)
    mask = (t > 0).astype(np.float64)
    cnt = np.bincount(dims, minlength=NI).astype(np.float64)
    sv = np.bincount(dims, weights=v, minlength=NI)
    corr = np.concatenate([EPS * (cnt + 1.0), EPS * sv]).astype(np.float32)
    corr = corr.reshape(P, 1)

    # host-precomputed stationary weights [N, 128] in bf16
    comb = np.zeros((N, 2 * NI), np.float32)
    rows = np.arange(N)
    comb[rows, dims] = mask
    comb[rows, NI + dims] = mask * v
    comb = comb.astype(ml_dtypes.bfloat16)

    in_maps = []
    for i in range(M):
        in_maps.append(
            {
                "s": S[i * ND : (i + 1) * ND],
                "comb": comb[i * ND : (i + 1) * ND],
                "grid": grid.astype(np.float32),
                "grid2": (grid * grid).astype(np.float32),
                "rho": rho,
                "corr": corr,
                "wd": np.ascontiguousarray(
                    W[:, i * RS : (i + 1) * RS].astype(np.float32)
                ),
            }
        )

    if os.environ.get("BASS_SIM"):
        from concourse.bass_interp import MultiCoreSim

        sim = MultiCoreSim(nc, M)
        for i in range(M):
            for k, val in in_maps[i].items():
                sim.cores[i].tensor(k)[:] = val
        sim.simulate()
        out = np.concatenate(
            [np.array(sim.cores[i].tensor("out")).T for i in range(M)], axis=0
        )
        last_results = None
    else:
        from concourse.bass_utils import run_bass_kernel_spmd

        res = run_bass_kernel_spmd(
            nc,
            in_maps,
            list(range(M)),
            trace=bool(os.environ.get("BASS_TRACE")),
        )
        last_results = res
        out = np.concatenate(
            [np.asarray(res.results[i]["out"]).T for i in range(M)], axis=0
        )

    return np.ascontiguousarray(out).reshape(1, R, 3 * NI).astype(np.float32)
